# revision 17
# baseline (speedup 1.0000x reference)
"""Two-layer GAT kernel for 8 Trainium2 NeuronCores — v3.

v3 strategy (4-bin gathers, grouped blocks, in-place fp16 slot reduce):
  * Nodes degree-sorted, dealt round-robin to 8 cores; each core aggregates
    messages for its 6250 nodes (dst-sharded, no cross-core reduce).
  * Sharded front end computes each core's stripe of a fat-row table
    (fp16, 512B rows: [h(128) | asrc(4) | ones(4) | junk]); AllGather
    replicates it.  Layer 2 repeats with 256B fp16 rows
    [h2(32) | a2s | one | junk].
  * Per-edge rows fetched with dma_gather (SWDGE, int16 indices < 32768).
    FOUR index bins per block group make every edge 2-3 way assignable,
    minimizing the per-block slot maxima:
      A: rows [0, 32768)        stride 512B
      B: rows [17408, 50176)    stride 512B
      E: even rows (any)        stride 1024B (idx = row>>1)
      O: odd rows (any)         stride 1024B from +512B base
    Bin capacities per group are chosen by an exact small LP (Hall
    feasibility over the 16 bin subsets); blocks are grouped by a DP that
    trades gather fixed cost against slot padding.
  * Gathers are grouped (one gather per bin per group of 3-6 blocks) to
    amortize the ~1.2us SWDGE fixed cost, rotating 4 SWDGE queues.
  * Attention weights are multiplied into the gathered rows IN PLACE
    (fp16, 2x DVE rate), then a pairwise tree reduce folds slots per
    block; static "ones" columns produce the softmax denominator in the
    same reduce.  Padding slots point at dummy rows with asrc = -30000
    (exp == 0 in fp16).
"""

import sys

sys.path.insert(0, "/opt/trn_rl_repo")

import numpy as np

import concourse.bacc as bacc
import concourse.bass as bass
import concourse.mybir as mybir
import concourse.tile as tile
from concourse.bass_utils import run_bass_kernel_spmd

F32 = mybir.dt.float32
F16 = mybir.dt.float16
I16 = mybir.dt.int16
AL = mybir.AluOpType
ACT = mybir.ActivationFunctionType

CORES = 8
NEG_SLOPE = 0.2
NEG_BIG = -30000.0          # fp16-safe; exp(leaky(-30000)) == 0

# problem constants (nn_GAT_35296041238878)
N = 50000
IN_DIM = 128
HID = 32
HEADS = 4
OUT_DIM = 32

BPC = 49
STRIDE = BPC * 128           # 6272 table rows per core
TBL_ROWS = CORES * STRIDE    # 50176
BBASE = TBL_ROWS - 32768     # 17408, window-B base
NPC = N // CORES             # 6250 real nodes per core

# layer-1 row (fp16, 512B): [h(0:128) | asrc(128:132) | ones(132:136) | junk]
L1_ROW = 256
L1H = HEADS * HID            # 128
W1N = L1H + 2 * HEADS        # 136 matmul cols [h | asrc | adst]
# layer-2 row (fp16, 256B): [h2(0:32) | a2s(32) | one(33) | junk]
L2_ROW = 128
W2N = OUT_DIM + 2            # 34 matmul cols [h2 | a2s | a2d]

# AllGather chunk boundaries in local rows; the shared tables are
# chunk-major: [chunk0: cores x rows | chunk1 | chunk2] so each chunked
# AllGather writes a contiguous range.
CH_BOUNDS = [0, 3584, STRIDE]
NCH = len(CH_BOUNDS) - 1
# table order of chunks: dummy-bearing chunk last-written but FIRST in the
# table, so every bin (A < 32768, B >= 17408, even, odd) reaches a dummy
CH_ORDER = [1, 0]
CH_BASE = [0] * NCH
_acc = 0
for _k in CH_ORDER:
    CH_BASE[_k] = _acc
    _acc += CORES * (CH_BOUNDS[_k + 1] - CH_BOUNDS[_k])


def _tblrow_map():
    """local position (c*STRIDE + s) -> chunk-major table row."""
    m = np.empty(TBL_ROWS, dtype=np.int64)
    for c in range(CORES):
        for k in range(NCH):
            lo, hi = CH_BOUNDS[k], CH_BOUNDS[k + 1]
            m[c * STRIDE + lo:c * STRIDE + hi] = (
                CH_BASE[k] + (hi - lo) * c + np.arange(hi - lo))
    return m


NQ = 4                       # SWDGE queues
SMAX = 64                    # max slots per group tile (SBUF bound)
GROUP_FIX = 12               # DP: group fixed cost in slot units

_CACHE = {}

# ---------------------------------------------------------------------------
# Tile's DMASW lane round-robin is not SWDGE-queue-aware: partition the 8
# lanes so queue q uses lanes [q*2, q*2+2).
import concourse.tile_sem_assignment as _tsa


def _queue_aware_assign_tick(self, inst):
    q = getattr(inst, "queue_num", None)
    if q is not None and isinstance(inst, _tsa.DMAInst) \
            and inst.engine == _tsa.mybir.EngineType.Pool:
        if not hasattr(self, "_q_lane_ctr"):
            self._q_lane_ctr = {}
        ctr = self._q_lane_ctr.get(q, 0)
        self._q_lane_ctr[q] = ctr + 1
        lanes = max(1, self.swdge_sem_count // NQ)
        self.next_sw_dma_idx = (q % NQ) * lanes + (ctr % lanes)
    return _tsa.TileClockTick._orig_assign_tick(self, inst)


if not hasattr(_tsa.TileClockTick, "_orig_assign_tick"):
    _tsa.TileClockTick._orig_assign_tick = _tsa.TileClockTick._assign_tick
    _tsa.TileClockTick._assign_tick = _queue_aware_assign_tick


# ---------------------------------------------------------------------------
# host-side graph preprocessing
# ---------------------------------------------------------------------------
_COPT = np.array([1 | 4, 1 | 8, 2 | 4, 2 | 8, 1 | 2 | 4, 1 | 2 | 8])
_SUBSEL = [[(int(_COPT[c]) & ~S) == 0 for c in range(6)] for S in range(16)]


def _opt_caps(M):
    """Min DA+DB+DE+DO s.t. sum over S-subset >= M[S] for all 16 subsets."""
    for T in range(int(M[15]), int(M[15]) + 16):
        for DA in range(int(M[1]), T + 1):
            for DB in range(int(M[2]), T - DA + 1):
                for DE in range(int(M[4]), T - DA - DB + 1):
                    DO = T - DA - DB - DE
                    if DO < M[8]:
                        continue
                    D = (DA, DB, DE, DO)
                    ok = True
                    for S in range(16):
                        cap = 0
                        for i in range(4):
                            if S >> i & 1:
                                cap += D[i]
                        if cap < M[S]:
                            ok = False
                            break
                    if ok:
                        return T, D
    raise RuntimeError("cap search failed")


def _assign_bins(C, caps):
    """Per-node edge->bin counts. C: [n,6] class counts (AE,AO,BE,BO,FE,FO).
    Returns [n,6] arrays: for each class, how many go to the PARITY bin
    (classes 0-3) / how many flex go to A (classes 4,5 -> (toE/toO, toA)).
    Output: eAE,oAO,eBE,oBO (fixed classes sent to parity), fEe,fOo
    (flex sent to parity), fA (flex sent to A; rest of flex goes to B)."""
    DA, DB, DE, DO = caps
    cAE, cAO, cBE, cBO, cFE, cFO = [C[:, i].astype(np.int64) for i in range(6)]
    needA = np.maximum(0, cAE + cAO - DA)
    needB = np.maximum(0, cBE + cBO - DB)
    eAE = np.minimum(cAE, needA)
    oAO = needA - eAE
    eBE = np.minimum(cBE, needB)
    oBO = needB - eBE
    # parity cap overflow: shift A-relief / B-relief between E and O
    for _ in range(2):
        xsE = np.maximum(0, (eAE + eBE) - DE)
        # shift AE->AO
        s1 = np.minimum(np.minimum(eAE, cAO - oAO), xsE)
        eAE -= s1
        oAO += s1
        xsE -= s1
        # shift BE->BO
        s2 = np.minimum(np.minimum(eBE, cBO - oBO), xsE)
        eBE -= s2
        oBO += s2
        xsO = np.maximum(0, (oAO + oBO) - DO)
        s3 = np.minimum(np.minimum(oAO, cAE - eAE), xsO)
        oAO -= s3
        eAE += s3
        xsO -= s3
        s4 = np.minimum(np.minimum(oBO, cBE - eBE), xsO)
        oBO -= s4
        eBE += s4
    # flex placement: parity first, then A, then B
    slackE = DE - (eAE + eBE)
    slackO = DO - (oAO + oBO)
    slackA = DA - (cAE - eAE + cAO - oAO)
    slackB = DB - (cBE - eBE + cBO - oBO)
    fEe = np.minimum(cFE, np.maximum(0, slackE))
    fOo = np.minimum(cFO, np.maximum(0, slackO))
    rest = (cFE - fEe) + (cFO - fOo)
    fA = np.minimum(rest, np.maximum(0, slackA))
    # remainder goes to B implicitly
    loads = np.stack([
        cAE - eAE + cAO - oAO + fA,
        cBE - eBE + cBO - oBO + (rest - fA),
        eAE + eBE + fEe,
        oAO + oBO + fOo,
    ], axis=1)
    return (eAE, oAO, eBE, oBO, fEe, fOo, fA), loads


def _assign_node_flow(counts, caps):
    """Exact per-node assignment by BFS augmenting paths. counts: [6]."""
    cap = list(caps)
    load = [0, 0, 0, 0]
    opts = [(0, 2), (0, 3), (1, 2), (1, 3), (0, 1, 2), (0, 1, 3)]
    assign = [[0] * 4 for _ in range(6)]
    for c in range(6):
        for _ in range(int(counts[c])):
            # BFS from c's option bins to any bin with spare capacity,
            # relocating along the path
            prev = {b: (None, None) for b in opts[c]}
            frontier = list(opts[c])
            target = None
            while frontier and target is None:
                nxt = []
                for b in frontier:
                    if load[b] < cap[b]:
                        target = b
                        break
                    for c2 in range(6):
                        if assign[c2][b] == 0:
                            continue
                        for b2 in opts[c2]:
                            if b2 not in prev:
                                prev[b2] = (b, c2)
                                nxt.append(b2)
                frontier = nxt
            if target is None:
                raise RuntimeError("node assignment infeasible")
            # walk back relocating
            b = target
            while prev[b][0] is not None:
                pb, pc = prev[b]
                assign[pc][pb] -= 1
                assign[pc][b] += 1
                b = pb
            assign[c][b] += 1
            load[target] += 1
    return assign


def _prep_graph(edge_index, n_nodes):
    src = np.concatenate([edge_index[0], np.arange(n_nodes)]).astype(np.int64)
    dst = np.concatenate([edge_index[1], np.arange(n_nodes)]).astype(np.int64)

    deg = np.bincount(dst, minlength=n_nodes)
    order = np.argsort(-deg, kind="stable")
    pos = np.empty(n_nodes, dtype=np.int64)
    ranks = np.arange(n_nodes)
    pos[order] = (ranks % CORES) * STRIDE + ranks // CORES
    nodes_of_core = [order[c::CORES] for c in range(CORES)]

    dpos = pos[dst]
    trow = _tblrow_map()
    sp = trow[pos[src]]
    blk = (dpos % STRIDE) // 128

    Acap = sp < 32768
    Bcap = sp >= BBASE
    par = (sp & 1).astype(np.int64)
    flex = Acap & Bcap
    onlyA = Acap & ~Bcap
    cls = np.where(flex, 4 + par, np.where(onlyA, 0 + par, 2 + par))

    CC = np.zeros((TBL_ROWS, 6), dtype=np.int32)
    np.add.at(CC, (dpos, cls), 1)

    nodeblk = (np.arange(TBL_ROWS) % STRIDE) // 128
    Mb = np.zeros((BPC, 16), dtype=np.int64)
    for S in range(1, 16):
        sel = _SUBSEL[S]
        if any(sel):
            dem = CC[:, sel].sum(axis=1)
            for b in range(BPC):
                Mb[b, S] = dem[nodeblk == b].max()

    # caps for every candidate contiguous block group of size <= 8
    Tij = {}
    for i in range(BPC):
        M = np.zeros(16, dtype=np.int64)
        for j in range(i, min(i + 8, BPC)):
            M = np.maximum(M, Mb[j])
            Tij[(i, j)] = _opt_caps(M)

    # DP over group boundaries
    INF = 1 << 30
    dp = [INF] * (BPC + 1)
    dp[0] = 0
    parent = [0] * (BPC + 1)
    for j in range(1, BPC + 1):
        for i in range(max(0, j - 8), j):
            T, _ = Tij[(i, j - 1)]
            if T * (j - i) > SMAX:
                continue
            c = dp[i] + T * (j - i) + GROUP_FIX
            if c < dp[j]:
                dp[j] = c
                parent[j] = i
    groups = []
    j = BPC
    while j > 0:
        i = parent[j]
        groups.append((i, j - 1, Tij[(i, j - 1)][1]))
        j = i
    groups.reverse()

    # per-node bin assignment
    edge_bin = np.full(len(dst), -1, dtype=np.int8)
    gid_of_block = np.zeros(BPC, dtype=np.int64)
    for gi, (b0, b1, caps) in enumerate(groups):
        gid_of_block[b0:b1 + 1] = gi
    for gi, (b0, b1, caps) in enumerate(groups):
        node_mask = (nodeblk >= b0) & (nodeblk <= b1)
        nodes = np.flatnonzero(node_mask & (CC.sum(axis=1) > 0))
        C = CC[nodes]
        (eAE, oAO, eBE, oBO, fEe, fOo, fA), loads = _assign_bins(C, caps)
        capv = np.array(caps)
        bad = np.flatnonzero((loads > capv[None, :]).any(axis=1))
        # per-class -> bin counts [n, 6, 4]
        n = len(nodes)
        a = np.zeros((n, 6, 4), dtype=np.int64)
        a[:, 0, 2] = eAE
        a[:, 0, 0] = C[:, 0] - eAE
        a[:, 1, 3] = oAO
        a[:, 1, 0] = C[:, 1] - oAO
        a[:, 2, 2] = eBE
        a[:, 2, 1] = C[:, 2] - eBE
        a[:, 3, 3] = oBO
        a[:, 3, 1] = C[:, 3] - oBO
        a[:, 4, 2] = fEe
        a[:, 5, 3] = fOo
        restE = C[:, 4] - fEe
        restO = C[:, 5] - fOo
        fAe = np.minimum(restE, fA)
        a[:, 4, 0] = fAe
        a[:, 4, 1] = restE - fAe
        a[:, 5, 0] = fA - fAe
        a[:, 5, 1] = restO - (fA - fAe)
        for k in bad:
            a[k] = np.array(_assign_node_flow(C[k], caps))
        assert (a.sum(axis=1) <= capv[None, :]).all()
        assert (a.sum(axis=2) == C).all()
        # per (node, class): how many of that class go to each bin.
        # distribute actual edges: order edges of same (node, class)
        # arbitrarily, fill bins in option order
        CC_assign_nodes = nodes
        # store into a flat lookup: for edge e with node i, class c, its
        # rank r within (i, c) -> bin = smallest bin with cumulated cap
        # Build cumulative thresholds per (node, class): bins in fixed order
        # [A, B, E, O]
        thr = np.cumsum(a, axis=2)  # [n, 6, 4]
        # map from node id -> row in this group's arrays
        row_of = np.full(TBL_ROWS, -1, dtype=np.int64)
        row_of[nodes] = np.arange(n)
        em = node_mask[dpos]
        eidx = np.flatnonzero(em)
        enode = dpos[eidx]
        ecls = cls[eidx]
        # rank within (node, class)
        o2 = np.lexsort((ecls, enode))
        so = eidx[o2]
        key = dpos[so] * 6 + cls[so]
        chg = np.r_[True, key[1:] != key[:-1]]
        starts = np.flatnonzero(chg)
        gidv = np.cumsum(chg) - 1
        rank = np.arange(len(so)) - starts[gidv]
        r_ = row_of[dpos[so]]
        t = thr[r_, cls[so]]  # [m, 4]
        rk = rank[:, None]
        binv = (rk >= t).sum(axis=1)  # 0..3
        edge_bin[so] = binv.astype(np.int8)

    assert (edge_bin >= 0).all()

    # slot index within (node, bin)
    o3 = np.lexsort((edge_bin, dpos))
    so = o3
    key = dpos[so] * 4 + edge_bin[so]
    chg = np.r_[True, key[1:] != key[:-1]]
    starts = np.flatnonzero(chg)
    gidv = np.cumsum(chg) - 1
    slot = np.empty(len(so), dtype=np.int64)
    slot[so] = np.arange(len(so)) - starts[gidv]

    # dummy rows: every core's local rows [NPC, STRIDE)
    all_dmy = np.concatenate(
        [trow[c * STRIDE + NPC:(c + 1) * STRIDE] for c in range(CORES)])
    dmyA = int(all_dmy[all_dmy < 32768][0])
    dmyB = int(all_dmy[all_dmy >= BBASE][0]) - BBASE
    dmyE = int(all_dmy[all_dmy % 2 == 0][0]) >> 1
    dmyO = int(all_dmy[all_dmy % 2 == 1][0]) >> 1
    dmy_vals = [dmyA, dmyB, dmyE, dmyO]

    # build per-core index tables: one flat [128, totcols] int16
    ecore = dpos // STRIDE
    eblk = blk
    elane = (dpos % STRIDE) % 128
    ebin = edge_bin.astype(np.int64)
    # index value per bin
    ival = np.where(ebin == 0, sp,
                    np.where(ebin == 1, sp - BBASE, sp >> 1))
    assert (ival >= 0).all() and (ival < 32768).all()

    offs = []          # per (group, bin): slot-column offset
    off = 0
    for (b0, b1, caps) in groups:
        gsz = b1 - b0 + 1
        bo = []
        for x in range(4):
            bo.append(off)
            off += caps[x] * gsz
        offs.append(bo)
    tot_slots = off

    idx_tables = []
    for c in range(CORES):
        flat = np.empty(128 * tot_slots, dtype=np.int64)
        # fill dummies per gather region
        for gi, (b0, b1, caps) in enumerate(groups):
            gsz = b1 - b0 + 1
            for x in range(4):
                o0 = offs[gi][x]
                flat[128 * o0:128 * (o0 + caps[x] * gsz)] = dmy_vals[x]
        m = ecore == c
        gi_e = gid_of_block[eblk[m]]
        b0_e = np.array([g[0] for g in groups])[gi_e]
        gsz_e = np.array([g[1] - g[0] + 1 for g in groups])[gi_e]
        capmat = np.array([g[2] for g in groups])  # [ngroups, 4]
        Dx = capmat[gi_e, ebin[m]]
        off_e = np.array(offs)[gi_e, ebin[m]]
        col = off_e + slot[m] * gsz_e + (eblk[m] - b0_e)
        fpos = col * 128 + elane[m]
        assert len(np.unique(fpos)) == len(fpos)
        flat[fpos] = ival[m]
        wrapped = flat.reshape(-1, 16).T.astype(np.int16)  # [16, 8*tot]
        idx_tables.append(np.tile(wrapped, (8, 1)))        # [128, 8*tot]

    return dict(
        groups=groups, offs=offs, tot_slots=tot_slots,
        pos=pos, nodes_of_core=nodes_of_core,
        idx=idx_tables,
    )


# ---------------------------------------------------------------------------
# device program
# ---------------------------------------------------------------------------
def _build_program(groups, offs, tot_slots, has_b1):
    nc = bacc.Bacc("TRN2", target_bir_lowering=False, debug=False,
                   num_devices=CORES, num_swdge_queues=NQ,
                   dynamic_dma_scratch_size=32768)

    xTs = nc.dram_tensor("xTs", [128, STRIDE], F16, kind="ExternalInput")
    w1e = nc.dram_tensor("w1e", [128, W1N], F16, kind="ExternalInput")
    w2e = nc.dram_tensor("w2e", [L1H, W2N], F16, kind="ExternalInput")
    b1t = nc.dram_tensor("b1t", [128, L1H], F32, kind="ExternalInput")
    ident = nc.dram_tensor("ident", [128, 128], F32, kind="ExternalInput")
    idxt = nc.dram_tensor("idxt", [128, 8 * tot_slots], I16,
                          kind="ExternalInput")

    cc1 = nc.dram_tensor("cc1", [STRIDE, L1_ROW], F16)
    tbl1 = nc.dram_tensor("tbl1", [TBL_ROWS, L1_ROW], F16, addr_space="Shared")
    cc2 = nc.dram_tensor("cc2", [STRIDE, L2_ROW], F16)
    tbl2 = nc.dram_tensor("tbl2", [TBL_ROWS, L2_ROW], F16, addr_space="Shared")
    out = nc.dram_tensor("out", [STRIDE, OUT_DIM], F32, kind="ExternalOutput")

    with tile.TileContext(nc) as tc:
        with (
            tc.tile_pool(name="res", bufs=1) as res,
            tc.tile_pool(name="ps", bufs=2, space="PSUM") as psp,
            tc.tile_pool(name="sml", bufs=2) as sml,
        ):
            # ---- resident constants ----
            w1e_t = res.tile([128, W1N], F16, tag="w1e")
            nc.sync.dma_start(w1e_t[:], w1e.ap())
            w2e_t = res.tile([L1H, W2N], F16, tag="w2e")
            nc.sync.dma_start(w2e_t[:], w2e.ap())
            b1_t = res.tile([128, L1H], F32, tag="b1")
            nc.sync.dma_start(b1_t[:], b1t.ap())
            id_t = res.tile([128, 128], F32, tag="ident")
            nc.sync.dma_start(id_t[:], ident.ap())
            idx_t = res.tile([128, 8 * tot_slots], I16, tag="idx")
            nc.sync.dma_start(idx_t[:], idxt.ap())
            ad_own = res.tile([128, BPC * HEADS], F16, tag="adown")
            ad2_own = res.tile([128, BPC], F16, tag="ad2own")

            # dummy rows [NPC, STRIDE): h = 0, asrc = NEG_BIG
            pad_rows = STRIDE - NPC
            dmy1 = res.tile([pad_rows, L1_ROW], F16, tag="dmy1")
            nc.vector.memset(dmy1[:], 0.0)
            nc.vector.memset(dmy1[:, L1H:L1H + HEADS], NEG_BIG)
            nc.sync.dma_start(cc1.ap()[NPC:STRIDE, :], dmy1[:])
            dmy2 = res.tile([pad_rows, L2_ROW], F16, tag="dmy2")
            nc.vector.memset(dmy2[:], 0.0)
            nc.vector.memset(dmy2[:, OUT_DIM:OUT_DIM + 1], NEG_BIG)
            nc.sync.dma_start(cc2.ap()[NPC:STRIDE, :], dmy2[:])

            # ---- front end: this core's stripe of the fat-row table ----
            fe_ctx = tc.tile_pool(name="fe", bufs=3)
            fe = fe_ctx.__enter__()
            xts_t = res.tile([128, STRIDE], F16, tag="xts")
            nc.sync.dma_start(xts_t[:], xTs.ap())
            FCH = 7
            for i in range(3):
                f0 = fe.tile([128, FCH, L1_ROW], F16, tag="fat")
                nc.vector.memset(
                    f0[:, :, L1H + HEADS:L1H + 2 * HEADS], 1.0)
            ag1_done = 0
            for t0 in range(0, BPC, FCH):
                tn = min(FCH, BPC - t0)
                fat = fe.tile([128, FCH, L1_ROW], F16, tag="fat")
                for k in range(tn):
                    t = t0 + k
                    ps = psp.tile([128, W1N], F32, tag="feps")
                    nc.tensor.matmul(ps[:], xts_t[:, 128 * t:128 * (t + 1)],
                                     w1e_t[:], start=True, stop=True)
                    nc.scalar.activation(
                        fat[:, k, 0:L1H + HEADS], ps[:, 0:L1H + HEADS],
                        ACT.Copy)
                    nc.vector.tensor_copy(
                        ad_own[:, HEADS * t:HEADS * (t + 1)],
                        ps[:, L1H + HEADS:L1H + 2 * HEADS])
                nrows = min(128 * tn, NPC - 128 * t0)
                nfull = nrows // 128
                if nfull > 0:
                    nc.sync.dma_start(
                        cc1.ap()[128 * t0:128 * (t0 + nfull), :].rearrange(
                            "(t p) e -> p t e", p=128), fat[:, 0:nfull, :])
                rem = nrows - nfull * 128
                if rem > 0:
                    nc.sync.dma_start(
                        cc1.ap()[128 * (t0 + nfull):128 * (t0 + nfull) + rem,
                                 :], fat[0:rem, nfull, :])
                done_rows = 128 * (t0 + tn)
                while ag1_done < 2 and CH_BOUNDS[ag1_done + 1] <= done_rows:
                    r0, r1 = CH_BOUNDS[ag1_done], CH_BOUNDS[ag1_done + 1]
                    nc.gpsimd.collective_compute(
                        "AllGather", AL.bypass,
                        replica_groups=[list(range(CORES))],
                        ins=[cc1.ap()[r0:r1, :].opt()],
                        outs=[tbl1.ap()[CH_BASE[ag1_done]:
                                        CH_BASE[ag1_done + 1], :].opt()])
                    ag1_done += 1

            fe_ctx.__exit__(None, None, None)
            r0, r1 = CH_BOUNDS[2], CH_BOUNDS[3]
            nc.gpsimd.collective_compute(
                "AllGather", AL.bypass,
                replica_groups=[list(range(CORES))],
                ins=[cc1.ap()[r0:r1, :].opt()],
                outs=[tbl1.ap()[CH_BASE[2]:CH_BASE[3], :].opt()])
            tc.strict_bb_all_engine_barrier()

            # table views for the 4 bins
            t1A = tbl1.ap()[0:32768, :]
            t1B = tbl1.ap()[BBASE:TBL_ROWS, :]
            t1P = tbl1.ap().rearrange("(r two) e -> r (two e)", two=2)
            t1E = t1P[:, 0:L1_ROW]
            t1O = t1P[:, L1_ROW:2 * L1_ROW]

            def fire_gathers(gt, gi, caps, gsz, tA, tB, tE, tO, row):
                tabs = [tA, tB, tE, tO]
                steps = [row, row, 2 * row, 2 * row]
                qq = 0
                sofs = offs[gi][0]
                for x in range(4):
                    dn = caps[x] * gsz
                    if dn == 0:
                        continue
                    o0 = offs[gi][x] - sofs
                    nc.gpsimd.dma_gather(
                        gt[:, o0:o0 + dn, :], tabs[x],
                        idx_t[:, 8 * offs[gi][x]:8 * (offs[gi][x] + dn)],
                        128 * dn, 128 * dn, row,
                        elem_step=steps[x], single_packet=False,
                        queue_num=(2 * gi + qq) % NQ)
                    qq += 1

            def tree_fold(buf, o0, D, gsz, W):
                """Slot-major bin region: columns [o0, o0+D*gsz), fold the
                slot dim (outer) -> partial sums in columns [o0, o0+gsz)."""
                Dt = 1 << (D.bit_length() - 1)
                if Dt == D and D > 1:
                    Dt >>= 1
                if D > Dt:
                    k = D - Dt
                    nc.vector.tensor_tensor(
                        buf[:, o0:o0 + k * gsz, 0:W],
                        buf[:, o0:o0 + k * gsz, 0:W],
                        buf[:, o0 + Dt * gsz:o0 + D * gsz, 0:W], AL.add)
                k = Dt >> 1
                while k >= 1:
                    nc.vector.tensor_tensor(
                        buf[:, o0:o0 + k * gsz, 0:W],
                        buf[:, o0:o0 + k * gsz, 0:W],
                        buf[:, o0 + k * gsz:o0 + 2 * k * gsz, 0:W], AL.add)
                    k >>= 1

            # ---- layer 1 ----
            l1_gat_ctx = tc.tile_pool(name="gat1", bufs=3)
            gat = l1_gat_ctx.__enter__()
            l2f_pool_ctx = tc.tile_pool(name="l2f", bufs=2)
            l2fp = l2f_pool_ctx.__enter__()
            GMAX = max(b1 - b0 + 1 for b0, b1, _ in groups)
            for i in range(2):
                lf = l2fp.tile([128, GMAX, L2_ROW], F16, tag="l2f")
                nc.vector.memset(lf[:, :, OUT_DIM + 1:OUT_DIM + 2], 1.0)

            ag2_done = [0]

            for gi, (b0, b1, caps) in enumerate(groups):
                gsz = b1 - b0 + 1
                S_g = sum(caps) * gsz
                gt = gat.tile([128, S_g, L1_ROW], F16, tag="g")
                fire_gathers(gt, gi, caps, gsz, t1A, t1B, t1E, t1O, L1_ROW)

                # z = asrc + adst  (per bin, slot-major block-broadcast)
                z = sml.tile([128, S_g, HEADS], F16, tag="z")
                adb = ad_own[:, HEADS * b0:HEADS * (b1 + 1)].rearrange(
                    "p (g h) -> p g h", g=gsz)
                sofs = offs[gi][0]
                for x in range(4):
                    if caps[x] == 0:
                        continue
                    o0 = offs[gi][x] - sofs
                    dn = caps[x] * gsz
                    nc.vector.tensor_tensor(
                        z[:, o0:o0 + dn, :].rearrange(
                            "p (d g) h -> p d g h", g=gsz),
                        gt[:, o0:o0 + dn, L1H:L1H + HEADS].rearrange(
                            "p (d g) h -> p d g h", g=gsz),
                        adb.unsqueeze(1).broadcast_to(
                            [128, caps[x], gsz, HEADS]), AL.add)
                z2 = sml.tile([128, S_g, HEADS], F16, tag="z2")
                nc.vector.scalar_tensor_tensor(
                    z2[:].rearrange("p a b -> p (a b)"),
                    z[:].rearrange("p a b -> p (a b)"), NEG_SLOPE,
                    z[:].rearrange("p a b -> p (a b)"),
                    op0=AL.mult, op1=AL.max)
                wb = sml.tile([128, S_g, HEADS], F16, tag="wb")
                nc.scalar.activation(
                    wb[:].rearrange("p a b -> p (a b)"),
                    z2[:].rearrange("p a b -> p (a b)"), ACT.Exp)

                # in-place weight multiply over the whole 136-wide row:
                # k = 0..31 -> h (c-major interleave), k = 32 junk asrc,
                # k = 33 ones -> denominator.  Unit-stride fp16 everywhere.
                W = L1H + 2 * HEADS    # 136
                nc.vector.tensor_tensor(
                    gt[:, :, 0:W].rearrange("p s (k h) -> p s k h", h=HEADS),
                    gt[:, :, 0:W].rearrange("p s (k h) -> p s k h", h=HEADS),
                    wb[:].unsqueeze(2).broadcast_to(
                        [128, S_g, W // HEADS, HEADS]), AL.mult)

                # flat tree fold over the whole group region: slot-major
                # columns are (anything, g) so any gsz-aligned fold is valid
                tree_fold(gt, 0, sum(caps), gsz, W)
                r = sml.tile([128, GMAX, W], F16, tag="r")
                nc.vector.tensor_copy(r[:, 0:gsz, :], gt[:, 0:gsz, 0:W])

                rec = sml.tile([128, GMAX, HEADS], F32, tag="rec")
                nc.vector.reciprocal(
                    rec[:, 0:gsz, :],
                    r[:, 0:gsz, L1H + HEADS:L1H + 2 * HEADS])
                o1 = sml.tile([128, GMAX, L1H], F32, tag="o1")
                nc.vector.tensor_tensor(
                    o1[:, 0:gsz, :].rearrange("p g (c h) -> p g c h", h=HEADS),
                    r[:, 0:gsz, 0:L1H].rearrange("p g (c h) -> p g c h",
                                                 h=HEADS),
                    rec[:, 0:gsz, :].unsqueeze(2).broadcast_to(
                        [128, gsz, HID, HEADS]), AL.mult)
                if has_b1:
                    nc.vector.tensor_tensor(
                        o1[:, 0:gsz, :], o1[:, 0:gsz, :],
                        b1_t[:].unsqueeze(1).broadcast_to([128, gsz, L1H]),
                        AL.add)
                # elu(x) = max(x, exp(min(x, 0)) - 1); min via relu(-x) on ACT
                e1n = sml.tile([128, GMAX, L1H], F32, tag="e1n")
                nc.scalar.activation(
                    e1n[:, 0:gsz, :], o1[:, 0:gsz, :], ACT.Relu, scale=-1.0)
                e2 = sml.tile([128, GMAX, L1H], F32, tag="e2")
                nc.scalar.activation(
                    e2[:, 0:gsz, :], e1n[:, 0:gsz, :], ACT.Exp, scale=-1.0)
                elu = sml.tile([128, GMAX, L1H], F32, tag="elu")
                nc.vector.scalar_tensor_tensor(
                    elu[:, 0:gsz, :], e2[:, 0:gsz, :], -1.0,
                    o1[:, 0:gsz, :], op0=AL.add, op1=AL.max)

                # layer-2 fat rows: h2' = elu^T @ W2ext per block
                l2fat = l2fp.tile([128, GMAX, L2_ROW], F16, tag="l2f")
                for k in range(gsz):
                    b = b0 + k
                    tp = psp.tile([128, 128], F32, tag="tp")
                    nc.tensor.transpose(tp[:], elu[:, k, :], id_t[:])
                    eluT = sml.tile([128, 128], F16, tag="eluT")
                    nc.scalar.activation(eluT[:], tp[:], ACT.Copy)
                    h2p = psp.tile([128, W2N], F32, tag="h2p")
                    nc.tensor.matmul(h2p[:], eluT[:], w2e_t[:],
                                     start=True, stop=True)
                    nc.scalar.activation(
                        l2fat[:, k, 0:OUT_DIM + 1], h2p[:, 0:OUT_DIM + 1],
                        ACT.Copy)
                    nc.vector.tensor_copy(
                        ad2_own[:, b:b + 1], h2p[:, OUT_DIM + 1:OUT_DIM + 2])
                nrows = min(128 * gsz, NPC - 128 * b0)
                nfull = nrows // 128
                if nfull > 0:
                    nc.sync.dma_start(
                        cc2.ap()[128 * b0:128 * (b0 + nfull), :].rearrange(
                            "(t p) e -> p t e", p=128), l2fat[:, 0:nfull, :])
                rem = nrows - nfull * 128
                if rem > 0:
                    nc.sync.dma_start(
                        cc2.ap()[128 * (b0 + nfull):128 * (b0 + nfull) + rem,
                                 :], l2fat[0:rem, nfull, :])
                done_rows = 128 * (b1 + 1)
                while ag2_done[0] < 2 and \
                        CH_BOUNDS[ag2_done[0] + 1] <= done_rows:
                    k = ag2_done[0]
                    r0, r1 = CH_BOUNDS[k], CH_BOUNDS[k + 1]
                    nc.gpsimd.collective_compute(
                        "AllGather", AL.bypass,
                        replica_groups=[list(range(CORES))],
                        ins=[cc2.ap()[r0:r1, :].opt()],
                        outs=[tbl2.ap()[CH_BASE[k]:CH_BASE[k + 1], :].opt()])
                    ag2_done[0] += 1

            l2f_pool_ctx.__exit__(None, None, None)
            l1_gat_ctx.__exit__(None, None, None)
            r0, r1 = CH_BOUNDS[2], CH_BOUNDS[3]
            nc.gpsimd.collective_compute(
                "AllGather", AL.bypass,
                replica_groups=[list(range(CORES))],
                ins=[cc2.ap()[r0:r1, :].opt()],
                outs=[tbl2.ap()[CH_BASE[2]:CH_BASE[3], :].opt()])
            tc.strict_bb_all_engine_barrier()

            # ---- layer 2 ----
            t2A = tbl2.ap()[0:32768, :]
            t2B = tbl2.ap()[BBASE:TBL_ROWS, :]
            t2P = tbl2.ap().rearrange("(r two) e -> r (two e)", two=2)
            t2E = t2P[:, 0:L2_ROW]
            t2O = t2P[:, L2_ROW:2 * L2_ROW]
            W2R = OUT_DIM + 2      # reduce width: [wh2 | junk a2s | wsum]

            l2_gat_ctx = tc.tile_pool(name="gat2", bufs=3)
            gat = l2_gat_ctx.__enter__()
            for gi, (b0, b1, caps) in enumerate(groups):
                gsz = b1 - b0 + 1
                S_g = sum(caps) * gsz
                g2 = gat.tile([128, S_g, L2_ROW], F16, tag="g2")
                fire_gathers(g2, gi, caps, gsz, t2A, t2B, t2E, t2O, L2_ROW)

                z = sml.tile([128, S_g], F16, tag="z2l")
                ad2b = ad2_own[:, b0:b1 + 1]
                sofs = offs[gi][0]
                for x in range(4):
                    if caps[x] == 0:
                        continue
                    o0 = offs[gi][x] - sofs
                    nc.vector.tensor_tensor(
                        z[:, o0:o0 + caps[x] * gsz].rearrange(
                            "p (d g) -> p d g", g=gsz),
                        g2[:, o0:o0 + caps[x] * gsz, OUT_DIM].rearrange(
                            "p (d g) -> p d g", g=gsz),
                        ad2b.unsqueeze(1).broadcast_to([128, caps[x], gsz]),
                        AL.add)
                z2 = sml.tile([128, S_g], F16, tag="z2l2")
                nc.vector.scalar_tensor_tensor(
                    z2[:, :], z[:, :], NEG_SLOPE, z[:, :],
                    op0=AL.mult, op1=AL.max)
                # duplicated weight pair for unit-stride broadcast multiply
                wp = sml.tile([128, S_g, 2], F16, tag="wp")
                nc.scalar.activation(wp[:, :, 0], z2[:, :], ACT.Exp)
                nc.scalar.activation(wp[:, :, 1], z2[:, :], ACT.Exp)

                # single in-place multiply over cols [0:34]:
                # [h2(32) | junk a2s | one] as (k, 2) pairs
                nc.vector.tensor_tensor(
                    g2[:, :, 0:W2R].rearrange("p s (k t) -> p s k t", t=2),
                    g2[:, :, 0:W2R].rearrange("p s (k t) -> p s k t", t=2),
                    wp[:].unsqueeze(2).broadcast_to(
                        [128, S_g, W2R // 2, 2]), AL.mult)

                tree_fold(g2, 0, sum(caps), gsz, W2R)
                r = sml.tile([128, GMAX, W2R], F16, tag="r2")
                nc.vector.tensor_copy(r[:, 0:gsz, :], g2[:, 0:gsz, 0:W2R])

                rec = sml.tile([128, GMAX], F32, tag="rec2")
                nc.vector.reciprocal(
                    rec[:, 0:gsz], r[:, 0:gsz, OUT_DIM + 1])
                o2 = sml.tile([128, GMAX, OUT_DIM], F32, tag="o2")
                nc.vector.tensor_tensor(
                    o2[:, 0:gsz, :], r[:, 0:gsz, 0:OUT_DIM],
                    rec[:, 0:gsz].unsqueeze(2).broadcast_to(
                        [128, gsz, OUT_DIM]), AL.mult)
                nrows = min(128 * gsz, NPC - 128 * b0)
                nfull = nrows // 128
                if nfull > 0:
                    nc.sync.dma_start(
                        out.ap()[128 * b0:128 * (b0 + nfull), :].rearrange(
                            "(t p) e -> p t e", p=128), o2[:, 0:nfull, :])
                rem = nrows - nfull * 128
                if rem > 0:
                    nc.sync.dma_start(
                        out.ap()[128 * (b0 + nfull):128 * (b0 + nfull) + rem,
                                 :], o2[0:rem, nfull, :])

            l2_gat_ctx.__exit__(None, None, None)

    nc.compile()
    return nc


# ---------------------------------------------------------------------------
# weight prep + end-to-end run
# ---------------------------------------------------------------------------
def _run(x, edge_index, W1, a1_src, a1_dst, b1, W2, a2_src, a2_dst, b2,
         trace=False, n_nodes=None, bpc=None):
    x = np.asarray(x, dtype=np.float32)
    edge_index = np.asarray(edge_index)

    g = _prep_graph(edge_index, N)

    has_b1 = bool(np.abs(np.asarray(b1)).max() > 0)
    key = (5, has_b1, tuple((b0, b1, tuple(c)) for b0, b1, c in g["groups"]))
    if key in _CACHE:
        nc = _CACHE[key]
    else:
        nc = _build_program(g["groups"], g["offs"], g["tot_slots"], has_b1)
        _CACHE[key] = nc

    W1 = np.asarray(W1, np.float32)
    W2 = np.asarray(W2, np.float32)
    w1s = np.stack([W1[:, h * HID:(h + 1) * HID]
                    @ np.asarray(a1_src, np.float32)[h]
                    for h in range(HEADS)], axis=1)
    w1d = np.stack([W1[:, h * HID:(h + 1) * HID]
                    @ np.asarray(a1_dst, np.float32)[h]
                    for h in range(HEADS)], axis=1)
    W1i = W1.reshape(IN_DIM, HEADS, HID).transpose(0, 2, 1).reshape(
        IN_DIM, L1H)              # h cols in (c, h) interleave
    w1e_np = np.concatenate([W1i, w1s, w1d], axis=1)
    w2s = (W2 @ np.asarray(a2_src, np.float32)[0])[:, None]
    w2d = (W2 @ np.asarray(a2_dst, np.float32)[0])[:, None]
    w2e_np = np.concatenate([W2, w2s, w2d], axis=1)
    # rows of w2e follow the (c, h) interleave of layer-1 features
    w2e_np = w2e_np.reshape(HEADS, HID, W2N).transpose(1, 0, 2).reshape(
        L1H, W2N)

    xT = np.zeros((IN_DIM, TBL_ROWS), dtype=np.float32)
    xT[:, g["pos"]] = x.T

    common = {
        "w1e": w1e_np.astype(np.float16),
        "w2e": w2e_np.astype(np.float16),
        "b1t": np.tile(np.asarray(b1, np.float32).reshape(
            HEADS, HID).T.reshape(1, L1H), (128, 1)),
        "ident": np.eye(128, dtype=np.float32),
    }
    in_maps = []
    for c in range(CORES):
        in_maps.append({
            **common,
            "xTs": xT[:, c * STRIDE:(c + 1) * STRIDE].astype(np.float16),
            "idxt": g["idx"][c],
        })

    res = run_bass_kernel_spmd(nc, in_maps, list(range(CORES)), trace=trace)

    out_full = np.empty((N, OUT_DIM), dtype=np.float32)
    for c in range(CORES):
        out_full[g["nodes_of_core"][c]] = res.results[c]["out"][0:NPC]
    out_full += np.asarray(b2, np.float32)[None, :]
    return out_full, res


def kernel(x, edge_index, W1, a1_src, a1_dst, b1, W2, a2_src, a2_dst, b2):
    out, _ = _run(x, edge_index, W1, a1_src, a1_dst, b1, W2, a2_src, a2_dst,
                  b2)
    return out
